# revision 9
# baseline (speedup 1.0000x reference)
"""Windowed cross-attention with relative position encodings, data-parallel
over batch across 8 NeuronCores.

Sharding (per spec hint): B=32 is split 4-per-core across the 8 cores;
the small q/kv/proj weights and the 169x1152 RPE table are replicated.
Windows are independent so attention needs no cross-device communication.

All einsums are rewritten as plain batched matmuls (lax.dot_general with
leading batch dims) so XLA-neuron lowers them to TensorE batched GEMMs
instead of gather loops.  The static RPE gather (169 -> [49,49] table) is
folded on the host into dense per-(h,i,c,j) tables, and matmul operands are
cast to bf16 (f32 accumulation) to double PE throughput.
"""

import numpy as np

import jax
import jax.numpy as jnp

WS = 7
NH = 12
DIM = 384
HD = DIM // NH
L = WS * WS
SCALE = HD ** (-0.5)
N_CORES = 8

BF = jnp.bfloat16


def _relative_position_index() -> np.ndarray:
    coords = np.stack(np.meshgrid(np.arange(WS), np.arange(WS), indexing="ij"))
    flat = coords.reshape(2, -1)
    rel = flat[:, :, None] - flat[:, None, :]
    rel = rel.transpose(1, 2, 0).copy()
    rel[:, :, 0] += WS - 1
    rel[:, :, 1] += WS - 1
    rel[:, :, 0] *= 2 * WS - 1
    return rel.sum(-1)  # [L, L] int


_RPI = _relative_position_index()


def _partition(t, b):
    # [b, 56, 56, DIM] -> [b*64, L, NH*HD] window-major tokens
    nh = 56 // WS
    t = t.reshape(b, nh, WS, nh, WS, DIM)
    t = t.transpose(0, 1, 3, 2, 4, 5)  # b, wi, wj, ih, iw, d
    return t.reshape(b * nh * nh, L, DIM)


def _unpartition(t, b):
    nh = 56 // WS
    t = t.reshape(b, nh, nh, WS, WS, DIM)
    t = t.transpose(0, 1, 3, 2, 4, 5)
    return t.reshape(b, 56, 56, DIM)


def _bmm(a, b, out_dtype=jnp.float32):
    # batched matmul over arbitrary leading dims; PSUM accumulates f32,
    # out_dtype only controls the copy-out precision
    return jax.lax.dot_general(
        a, b,
        dimension_numbers=(((a.ndim - 1,), (b.ndim - 2,)),
                           (tuple(range(a.ndim - 2)), tuple(range(b.ndim - 2)))),
        preferred_element_type=out_dtype,
    )


def _core_fn(x, context, q_w, q_b, kv_w, kv_b, proj_w, proj_b,
             k_rpe_t, q_rpe_t, v_rpe_t):
    """Per-core compute.  x, context: [b, 56, 56, DIM] bf16.

    k_rpe_t: [NH, L, HD, L]  (h, i, c, j)   -- already SCALE-free
    q_rpe_t: [NH, L, HD, L]  (h, j, c, i)   -- already * SCALE
    v_rpe_t: [NH, L, L, HD]  (h, i, j, c)
    """
    b = x.shape[0]
    bw = b * 64
    toks = b * 56 * 56

    # --- projections on FLAT tokens (position-independent), then ONE fused
    # 6D permute per tensor does window-partition + head-split together.
    # (SCALE is folded into q_w/q_b on the host.)
    q = (_bmm(x.reshape(toks, DIM), q_w) + q_b).astype(BF)
    kv = (_bmm(context.reshape(toks, DIM), kv_w) + kv_b).astype(BF)

    # [b, wi, ih, wj, iw, h, c]
    q6 = q.reshape(b, 8, WS, 8, WS, NH, HD)
    k6 = kv[:, :DIM].reshape(b, 8, WS, 8, WS, NH, HD)
    v6 = kv[:, DIM:].reshape(b, 8, WS, 8, WS, NH, HD)

    # head-major [NH, L, bw, HD]: (h, ih, iw, b, wi, wj, c)
    q = q6.transpose(5, 2, 4, 0, 1, 3, 6).reshape(NH, L, bw, HD)
    k = k6.transpose(5, 2, 4, 0, 1, 3, 6).reshape(NH, L, bw, HD)
    # token-major [bw, NH, L, HD]: (b, wi, wj, h, ih, iw, c)
    v = v6.transpose(0, 1, 3, 5, 2, 4, 6).reshape(bw, NH, L, HD)

    # --- attention logits (bf16 copy-out; PSUM still accumulates f32) ---
    # qk[h,b,i,j]: batch (h, b) taken in-place from [NH, L, bw, HD]
    qk = jax.lax.dot_general(
        q, k, (((3,), (3,)), ((0, 2), (0, 2))),
        preferred_element_type=BF)             # [NH, bw, L(i), L(j)]
    qk = qk.transpose(0, 2, 1, 3)              # [NH, L(i), bw, L(j)]

    # qr[h,i,b,j] = sum_c q[h,i,b,c] * k_rpe[h,i,c,j]  (native layout)
    qr = _bmm(q, k_rpe_t, BF)                  # [NH, L(i), bw, L(j)]

    # kr[h,j,b,i] = sum_c k[h,j,b,c] * q_rpe[h,j,c,i]
    kr = _bmm(k, q_rpe_t, BF)                  # [NH, L(j), bw, L(i)]
    kr = kr.transpose(0, 3, 2, 1)              # [NH, L(i), bw, L(j)]

    s = qk + qr + kr                           # bf16 [NH, L(i), bw, L(j)]

    # --- softmax over j (logits are small: skip max-subtraction) ---
    p = jnp.exp(s)                             # bf16
    z = jnp.sum(p, axis=-1, keepdims=True, dtype=jnp.float32)
    p = (p / z).astype(BF)                     # [NH, L(i), bw, L(j)]

    # --- values ---
    # o1[h,b,i,c] = sum_j p[h,i,b,j] v[b,h,j,c]
    o1 = jax.lax.dot_general(
        p, v, (((3,), (2,)), ((0, 2), (1, 0))),
        preferred_element_type=BF)             # [NH, bw, L(i), HD]

    # o2[h,i,b,c] = sum_j p[h,i,b,j] * v_rpe[h,i,j,c]  (p used in place)
    o2 = _bmm(p, v_rpe_t, BF)                  # [NH, L(i), bw, HD]

    # Fused un-partition + head-merge: one 6D permute per tensor brings
    # both straight to image token order (b, wi, ih, wj, iw, h, c), so the
    # projected output needs no separate unpartition.
    o1 = o1.reshape(NH, b, 8, 8, WS, WS, HD)   # (h, b, wi, wj, ih, iw, c)
    o1 = o1.transpose(1, 2, 4, 3, 5, 0, 6).reshape(toks, DIM)
    o2 = o2.reshape(NH, WS, WS, b, 8, 8, HD)   # (h, ih, iw, b, wi, wj, c)
    o2 = o2.transpose(3, 4, 1, 5, 2, 0, 6).reshape(toks, DIM)
    o = o1 + o2                                # bf16 [toks, DIM]

    # --- output projection ---
    y = _bmm(o, proj_w) + proj_b               # [toks, DIM] f32
    return y.reshape(b, 56, 56, DIM)


_PMAP = None


def _get_pmap():
    global _PMAP
    if _PMAP is None:
        _PMAP = jax.pmap(_core_fn, devices=jax.devices()[:N_CORES])
    return _PMAP


def _tile8(a):
    a = np.asarray(a)
    return np.broadcast_to(a, (N_CORES,) + a.shape)


def _prep_consts(rpe_table, q_w, q_b, kv_w, kv_b, proj_w, proj_b):
    # host-side fold of the static gather: [169, 1152] -> dense tables
    rpe = np.asarray(rpe_table)[_RPI.reshape(-1)].reshape(L, L, NH, 3 * HD)
    q_rpe, k_rpe, v_rpe = np.split(rpe, 3, axis=-1)   # [L(i), L(j), NH, HD]
    # k_rpe_t[h, i, c, j]
    k_rpe_t = k_rpe.transpose(2, 0, 3, 1).astype(ml_bf16())
    # q_rpe_t[h, j, c, i] (with SCALE folded in)
    q_rpe_t = (q_rpe * SCALE).transpose(2, 1, 3, 0).astype(ml_bf16())
    # v_rpe_t[h, i, j, c]
    v_rpe_t = v_rpe.transpose(2, 0, 1, 3).astype(ml_bf16())
    return dict(
        q_w=(np.asarray(q_w) * SCALE).astype(ml_bf16()),
        q_b=np.asarray(q_b, np.float32) * SCALE,
        kv_w=np.asarray(kv_w).astype(ml_bf16()),
        kv_b=np.asarray(kv_b, np.float32),
        proj_w=np.asarray(proj_w).astype(ml_bf16()),
        proj_b=np.asarray(proj_b, np.float32),
        k_rpe_t=k_rpe_t, q_rpe_t=q_rpe_t, v_rpe_t=v_rpe_t,
    )


def ml_bf16():
    import ml_dtypes
    return ml_dtypes.bfloat16


def kernel(x, context, rpe_table, q_w, q_b, kv_w, kv_b, proj_w, proj_b):
    x = np.asarray(x)
    context = np.asarray(context)
    B = x.shape[0]
    per = B // N_CORES

    consts = _prep_consts(rpe_table, q_w, q_b, kv_w, kv_b, proj_w, proj_b)

    xs = x.reshape(N_CORES, per, 56, 56, DIM).astype(ml_bf16())
    cs = context.reshape(N_CORES, per, 56, 56, DIM).astype(ml_bf16())

    out = _get_pmap()(
        xs, cs,
        _tile8(consts["q_w"]), _tile8(consts["q_b"]),
        _tile8(consts["kv_w"]), _tile8(consts["kv_b"]),
        _tile8(consts["proj_w"]), _tile8(consts["proj_b"]),
        _tile8(consts["k_rpe_t"]), _tile8(consts["q_rpe_t"]),
        _tile8(consts["v_rpe_t"]),
    )
    out = np.asarray(out).reshape(B, 56, 56, DIM)
    return out.astype(np.float32)


# revision 10
# speedup vs baseline: 3.5188x; 3.5188x over previous
"""Windowed cross-attention with relative position encodings, data-parallel
over batch across 8 NeuronCores.

Sharding (per spec hint): B=32 is split 4-per-core across the 8 cores;
the small q/kv/proj weights and the 169x1152 RPE table are replicated.
Windows are independent so attention needs no cross-device communication.

All einsums are rewritten as plain batched matmuls (lax.dot_general with
leading batch dims) so XLA-neuron lowers them to TensorE batched GEMMs
instead of gather loops.  The static RPE gather (169 -> [49,49] table) is
folded on the host into dense per-(h,i,c,j) tables, and matmul operands are
cast to bf16 (f32 accumulation) to double PE throughput.
"""

import numpy as np

import jax
import jax.numpy as jnp

WS = 7
NH = 12
DIM = 384
HD = DIM // NH
L = WS * WS
SCALE = HD ** (-0.5)
N_CORES = 8

BF = jnp.bfloat16


def _relative_position_index() -> np.ndarray:
    coords = np.stack(np.meshgrid(np.arange(WS), np.arange(WS), indexing="ij"))
    flat = coords.reshape(2, -1)
    rel = flat[:, :, None] - flat[:, None, :]
    rel = rel.transpose(1, 2, 0).copy()
    rel[:, :, 0] += WS - 1
    rel[:, :, 1] += WS - 1
    rel[:, :, 0] *= 2 * WS - 1
    return rel.sum(-1)  # [L, L] int


_RPI = _relative_position_index()


def _partition(t, b):
    # [b, 56, 56, DIM] -> [b*64, L, NH*HD] window-major tokens
    nh = 56 // WS
    t = t.reshape(b, nh, WS, nh, WS, DIM)
    t = t.transpose(0, 1, 3, 2, 4, 5)  # b, wi, wj, ih, iw, d
    return t.reshape(b * nh * nh, L, DIM)


def _unpartition(t, b):
    nh = 56 // WS
    t = t.reshape(b, nh, nh, WS, WS, DIM)
    t = t.transpose(0, 1, 3, 2, 4, 5)
    return t.reshape(b, 56, 56, DIM)


def _bmm(a, b, out_dtype=jnp.float32):
    # batched matmul over arbitrary leading dims; PSUM accumulates f32,
    # out_dtype only controls the copy-out precision
    return jax.lax.dot_general(
        a, b,
        dimension_numbers=(((a.ndim - 1,), (b.ndim - 2,)),
                           (tuple(range(a.ndim - 2)), tuple(range(b.ndim - 2)))),
        preferred_element_type=out_dtype,
    )


def _core_fn(x, context, q_w, q_b, kv_w, kv_b, proj_w, proj_b,
             k_rpe_t, q_rpe_t, v_rpe_t):
    """Per-core compute.  x, context: [b, 56, 56, DIM] bf16.

    k_rpe_t: [NH, L, HD, L]  (h, i, c, j)   -- already SCALE-free
    q_rpe_t: [NH, L, HD, L]  (h, j, c, i)   -- already * SCALE
    v_rpe_t: [NH, L, L, HD]  (h, i, j, c)
    """
    b = x.shape[0]
    bw = b * 64

    # --- projections (big dense GEMMs, bf16 x bf16 -> f32) ---
    xw = _partition(x, b)                      # [bw, L, DIM]
    cw = _partition(context, b)                # [bw, L, DIM]

    q = (_bmm(xw.reshape(bw * L, DIM), q_w) + q_b).astype(BF)      # [bw*L, DIM]
    kv = (_bmm(cw.reshape(bw * L, DIM), kv_w) + kv_b).astype(BF)   # [bw*L, 2*DIM]
    k = kv[:, :DIM]
    v = kv[:, DIM:]

    # Head-major token layout [NH, L, bw, HD] is primary: the RPE matmuls
    # (batched over (h, position)) consume and produce it natively, so q/k
    # are permuted once instead of twice and p feeds o2 with no transpose.
    # (SCALE is folded into q_w/q_b on the host.)
    q = q.reshape(bw, L, NH, HD).transpose(2, 1, 0, 3)   # [NH, L(i), bw, HD]
    k = k.reshape(bw, L, NH, HD).transpose(2, 1, 0, 3)   # [NH, L(j), bw, HD]
    v = v.reshape(bw, L, NH, HD).transpose(0, 2, 1, 3)   # [bw, NH, L(j), HD]

    # --- attention logits (bf16 copy-out; PSUM still accumulates f32) ---
    # qk[h,b,i,j]: batch (h, b) taken in-place from [NH, L, bw, HD]
    qk = jax.lax.dot_general(
        q, k, (((3,), (3,)), ((0, 2), (0, 2))),
        preferred_element_type=BF)             # [NH, bw, L(i), L(j)]
    qk = qk.transpose(0, 2, 1, 3)              # [NH, L(i), bw, L(j)]

    # qr[h,i,b,j] = sum_c q[h,i,b,c] * k_rpe[h,i,c,j]  (native layout)
    qr = _bmm(q, k_rpe_t, BF)                  # [NH, L(i), bw, L(j)]

    # kr[h,j,b,i] = sum_c k[h,j,b,c] * q_rpe[h,j,c,i]
    kr = _bmm(k, q_rpe_t, BF)                  # [NH, L(j), bw, L(i)]
    kr = kr.transpose(0, 3, 2, 1)              # [NH, L(i), bw, L(j)]

    s = qk + qr + kr                           # bf16 [NH, L(i), bw, L(j)]

    # --- softmax over j (logits are small: skip max-subtraction) ---
    p = jnp.exp(s)                             # bf16
    z = jnp.sum(p, axis=-1, keepdims=True, dtype=jnp.float32)
    p = (p / z).astype(BF)                     # [NH, L(i), bw, L(j)]

    # --- values ---
    # o1[h,b,i,c] = sum_j p[h,i,b,j] v[b,h,j,c]
    o1 = jax.lax.dot_general(
        p, v, (((3,), (2,)), ((0, 2), (1, 0))),
        preferred_element_type=BF)             # [NH, bw, L(i), HD]

    # o2[h,i,b,c] = sum_j p[h,i,b,j] * v_rpe[h,i,j,c]  (p used in place)
    o2 = _bmm(p, v_rpe_t, BF)                  # [NH, L(i), bw, HD]

    o = (o1.transpose(1, 2, 0, 3) + o2.transpose(2, 1, 0, 3))
    o = o.reshape(bw * L, DIM)                 # [bw*L, NH*HD] bf16

    # --- output projection ---
    y = _bmm(o, proj_w) + proj_b               # [bw*L, DIM] f32
    return _unpartition(y.reshape(bw, L, DIM), b)


_PMAP = None


def _get_pmap():
    global _PMAP
    if _PMAP is None:
        _PMAP = jax.pmap(_core_fn, devices=jax.devices()[:N_CORES])
    return _PMAP


def _tile8(a):
    a = np.asarray(a)
    return np.broadcast_to(a, (N_CORES,) + a.shape)


def _prep_consts(rpe_table, q_w, q_b, kv_w, kv_b, proj_w, proj_b):
    # host-side fold of the static gather: [169, 1152] -> dense tables
    rpe = np.asarray(rpe_table)[_RPI.reshape(-1)].reshape(L, L, NH, 3 * HD)
    q_rpe, k_rpe, v_rpe = np.split(rpe, 3, axis=-1)   # [L(i), L(j), NH, HD]
    # k_rpe_t[h, i, c, j]
    k_rpe_t = k_rpe.transpose(2, 0, 3, 1).astype(ml_bf16())
    # q_rpe_t[h, j, c, i] (with SCALE folded in)
    q_rpe_t = (q_rpe * SCALE).transpose(2, 1, 3, 0).astype(ml_bf16())
    # v_rpe_t[h, i, j, c]
    v_rpe_t = v_rpe.transpose(2, 0, 1, 3).astype(ml_bf16())
    return dict(
        q_w=(np.asarray(q_w) * SCALE).astype(ml_bf16()),
        q_b=np.asarray(q_b, np.float32) * SCALE,
        kv_w=np.asarray(kv_w).astype(ml_bf16()),
        kv_b=np.asarray(kv_b, np.float32),
        proj_w=np.asarray(proj_w).astype(ml_bf16()),
        proj_b=np.asarray(proj_b, np.float32),
        k_rpe_t=k_rpe_t, q_rpe_t=q_rpe_t, v_rpe_t=v_rpe_t,
    )


def ml_bf16():
    import ml_dtypes
    return ml_dtypes.bfloat16


def kernel(x, context, rpe_table, q_w, q_b, kv_w, kv_b, proj_w, proj_b):
    x = np.asarray(x)
    context = np.asarray(context)
    B = x.shape[0]
    per = B // N_CORES

    consts = _prep_consts(rpe_table, q_w, q_b, kv_w, kv_b, proj_w, proj_b)

    xs = x.reshape(N_CORES, per, 56, 56, DIM).astype(ml_bf16())
    cs = context.reshape(N_CORES, per, 56, 56, DIM).astype(ml_bf16())

    out = _get_pmap()(
        xs, cs,
        _tile8(consts["q_w"]), _tile8(consts["q_b"]),
        _tile8(consts["kv_w"]), _tile8(consts["kv_b"]),
        _tile8(consts["proj_w"]), _tile8(consts["proj_b"]),
        _tile8(consts["k_rpe_t"]), _tile8(consts["q_rpe_t"]),
        _tile8(consts["v_rpe_t"]),
    )
    out = np.asarray(out).reshape(B, 56, 56, DIM)
    return out.astype(np.float32)


# revision 11
# speedup vs baseline: 3.5206x; 1.0005x over previous
"""Windowed cross-attention with relative position encodings, data-parallel
over batch across 8 NeuronCores.

Sharding (per spec hint): B=32 is split 4-per-core across the 8 cores;
the small q/kv/proj weights and the 169x1152 RPE table are replicated.
Windows are independent so attention needs no cross-device communication.

All einsums are rewritten as plain batched matmuls (lax.dot_general with
leading batch dims) so XLA-neuron lowers them to TensorE batched GEMMs
instead of gather loops.  The static RPE gather (169 -> [49,49] table) is
folded on the host into dense per-(h,i,c,j) tables, and matmul operands are
cast to bf16 (f32 accumulation) to double PE throughput.
"""

import numpy as np

import jax
import jax.numpy as jnp

WS = 7
NH = 12
DIM = 384
HD = DIM // NH
L = WS * WS
SCALE = HD ** (-0.5)
N_CORES = 8

BF = jnp.bfloat16


def _relative_position_index() -> np.ndarray:
    coords = np.stack(np.meshgrid(np.arange(WS), np.arange(WS), indexing="ij"))
    flat = coords.reshape(2, -1)
    rel = flat[:, :, None] - flat[:, None, :]
    rel = rel.transpose(1, 2, 0).copy()
    rel[:, :, 0] += WS - 1
    rel[:, :, 1] += WS - 1
    rel[:, :, 0] *= 2 * WS - 1
    return rel.sum(-1)  # [L, L] int


_RPI = _relative_position_index()


def _partition(t, b):
    # [b, 56, 56, DIM] -> [b*64, L, NH*HD] window-major tokens
    nh = 56 // WS
    t = t.reshape(b, nh, WS, nh, WS, DIM)
    t = t.transpose(0, 1, 3, 2, 4, 5)  # b, wi, wj, ih, iw, d
    return t.reshape(b * nh * nh, L, DIM)


def _unpartition(t, b):
    nh = 56 // WS
    t = t.reshape(b, nh, nh, WS, WS, DIM)
    t = t.transpose(0, 1, 3, 2, 4, 5)
    return t.reshape(b, 56, 56, DIM)


def _bmm(a, b, out_dtype=jnp.float32):
    # batched matmul over arbitrary leading dims; PSUM accumulates f32,
    # out_dtype only controls the copy-out precision
    return jax.lax.dot_general(
        a, b,
        dimension_numbers=(((a.ndim - 1,), (b.ndim - 2,)),
                           (tuple(range(a.ndim - 2)), tuple(range(b.ndim - 2)))),
        preferred_element_type=out_dtype,
    )


def _core_fn(x, context, q_w, q_b, kv_w, kv_b, proj_w, proj_b,
             k_rpe_t, q_rpe_t, v_rpe_t):
    """Per-core compute.  x, context: [b, 56, 56, DIM] bf16.

    k_rpe_t: [NH, L, HD, L]  (h, i, c, j)   -- already SCALE-free
    q_rpe_t: [NH, L, HD, L]  (h, j, c, i)   -- already * SCALE
    v_rpe_t: [NH, L, L, HD]  (h, i, j, c)
    """
    b = x.shape[0]
    bw = b * 64

    # --- projections (big dense GEMMs, bf16 x bf16 -> f32) ---
    xw = _partition(x, b)                      # [bw, L, DIM]
    cw = _partition(context, b)                # [bw, L, DIM]

    # bf16 copy-out of the projection PSUM halves the bytes written; the
    # bias-adds run in bf16 (biases are ~0-scale, far inside the 2e-2 gate)
    q = _bmm(xw.reshape(bw * L, DIM), q_w, BF) + q_b.astype(BF)    # [bw*L, DIM]
    kv = _bmm(cw.reshape(bw * L, DIM), kv_w, BF) + kv_b.astype(BF)  # [bw*L, 2*DIM]
    k = kv[:, :DIM]
    v = kv[:, DIM:]

    # Head-major token layout [NH, L, bw, HD] is primary: the RPE matmuls
    # (batched over (h, position)) consume and produce it natively, so q/k
    # are permuted once instead of twice and p feeds o2 with no transpose.
    # (SCALE is folded into q_w/q_b on the host.)
    q = q.reshape(bw, L, NH, HD).transpose(2, 1, 0, 3)   # [NH, L(i), bw, HD]
    k = k.reshape(bw, L, NH, HD).transpose(2, 1, 0, 3)   # [NH, L(j), bw, HD]
    v = v.reshape(bw, L, NH, HD).transpose(0, 2, 1, 3)   # [bw, NH, L(j), HD]

    # --- attention logits (bf16 copy-out; PSUM still accumulates f32) ---
    # qk[h,b,i,j]: batch (h, b) taken in-place from [NH, L, bw, HD]
    qk = jax.lax.dot_general(
        q, k, (((3,), (3,)), ((0, 2), (0, 2))),
        preferred_element_type=BF)             # [NH, bw, L(i), L(j)]
    qk = qk.transpose(0, 2, 1, 3)              # [NH, L(i), bw, L(j)]

    # qr[h,i,b,j] = sum_c q[h,i,b,c] * k_rpe[h,i,c,j]  (native layout)
    qr = _bmm(q, k_rpe_t, BF)                  # [NH, L(i), bw, L(j)]

    # kr[h,j,b,i] = sum_c k[h,j,b,c] * q_rpe[h,j,c,i]
    kr = _bmm(k, q_rpe_t, BF)                  # [NH, L(j), bw, L(i)]
    kr = kr.transpose(0, 3, 2, 1)              # [NH, L(i), bw, L(j)]

    s = qk + qr + kr                           # bf16 [NH, L(i), bw, L(j)]

    # --- softmax over j (logits are small: skip max-subtraction) ---
    p = jnp.exp(s)                             # bf16
    z = jnp.sum(p, axis=-1, keepdims=True, dtype=jnp.float32)
    p = (p / z).astype(BF)                     # [NH, L(i), bw, L(j)]

    # --- values ---
    # o1[h,b,i,c] = sum_j p[h,i,b,j] v[b,h,j,c]
    o1 = jax.lax.dot_general(
        p, v, (((3,), (2,)), ((0, 2), (1, 0))),
        preferred_element_type=BF)             # [NH, bw, L(i), HD]

    # o2[h,i,b,c] = sum_j p[h,i,b,j] * v_rpe[h,i,j,c]  (p used in place)
    o2 = _bmm(p, v_rpe_t, BF)                  # [NH, L(i), bw, HD]

    o = (o1.transpose(1, 2, 0, 3) + o2.transpose(2, 1, 0, 3))
    o = o.reshape(bw * L, DIM)                 # [bw*L, NH*HD] bf16

    # --- output projection ---
    y = _bmm(o, proj_w) + proj_b               # [bw*L, DIM] f32
    return _unpartition(y.reshape(bw, L, DIM), b)


_PMAP = None


def _get_pmap():
    global _PMAP
    if _PMAP is None:
        _PMAP = jax.pmap(_core_fn, devices=jax.devices()[:N_CORES])
    return _PMAP


def _tile8(a):
    a = np.asarray(a)
    return np.broadcast_to(a, (N_CORES,) + a.shape)


def _prep_consts(rpe_table, q_w, q_b, kv_w, kv_b, proj_w, proj_b):
    # host-side fold of the static gather: [169, 1152] -> dense tables
    rpe = np.asarray(rpe_table)[_RPI.reshape(-1)].reshape(L, L, NH, 3 * HD)
    q_rpe, k_rpe, v_rpe = np.split(rpe, 3, axis=-1)   # [L(i), L(j), NH, HD]
    # k_rpe_t[h, i, c, j]
    k_rpe_t = k_rpe.transpose(2, 0, 3, 1).astype(ml_bf16())
    # q_rpe_t[h, j, c, i] (with SCALE folded in)
    q_rpe_t = (q_rpe * SCALE).transpose(2, 1, 3, 0).astype(ml_bf16())
    # v_rpe_t[h, i, j, c]
    v_rpe_t = v_rpe.transpose(2, 0, 1, 3).astype(ml_bf16())
    return dict(
        q_w=(np.asarray(q_w) * SCALE).astype(ml_bf16()),
        q_b=np.asarray(q_b, np.float32) * SCALE,
        kv_w=np.asarray(kv_w).astype(ml_bf16()),
        kv_b=np.asarray(kv_b, np.float32),
        proj_w=np.asarray(proj_w).astype(ml_bf16()),
        proj_b=np.asarray(proj_b, np.float32),
        k_rpe_t=k_rpe_t, q_rpe_t=q_rpe_t, v_rpe_t=v_rpe_t,
    )


def ml_bf16():
    import ml_dtypes
    return ml_dtypes.bfloat16


def kernel(x, context, rpe_table, q_w, q_b, kv_w, kv_b, proj_w, proj_b):
    x = np.asarray(x)
    context = np.asarray(context)
    B = x.shape[0]
    per = B // N_CORES

    consts = _prep_consts(rpe_table, q_w, q_b, kv_w, kv_b, proj_w, proj_b)

    xs = x.reshape(N_CORES, per, 56, 56, DIM).astype(ml_bf16())
    cs = context.reshape(N_CORES, per, 56, 56, DIM).astype(ml_bf16())

    out = _get_pmap()(
        xs, cs,
        _tile8(consts["q_w"]), _tile8(consts["q_b"]),
        _tile8(consts["kv_w"]), _tile8(consts["kv_b"]),
        _tile8(consts["proj_w"]), _tile8(consts["proj_b"]),
        _tile8(consts["k_rpe_t"]), _tile8(consts["q_rpe_t"]),
        _tile8(consts["v_rpe_t"]),
    )
    out = np.asarray(out).reshape(B, 56, 56, DIM)
    return out.astype(np.float32)
